# revision 24
# baseline (speedup 1.0000x reference)
"""Trainium2 Bass kernel for masked multi-head attention scores (softmax(QK^T)).

Reference computation (B=2, S=2048, D=768, H=12, DK=64):
    q = (query @ Wq.T + bq)  -> [B,H,S,DK]
    k = (key   @ Wk.T + bk)  -> [B,H,S,DK]
    scores = q @ k.T / sqrt(DK)            [B,H,S,S]
    m = where(mask == -10000, 1e9, 0)      [B,S]
    scores = scores - m[:,None,:,None] - m[:,None,None,:]
    out = softmax(scores, axis=-1)

Sharding: 8 cores = 2 batches x 4 head-groups (3 heads each). Each core gets
its batch's query^T/key^T (pre-transposed on host - pure layout change), its
3 heads' weight slices, and computes softmax scores for those heads.

Device algorithm per core:
  - Projections into [dk, s] layout via PE matmuls; heads 0+1 are packed
    into one M=128 matmul (they share the same rhs); 1/sqrt(DK) is folded
    into Wq/bq as an exact power-of-2 scale. K side runs first (scores need
    all of k); the Q side streams per 512-column slice, and the scores for
    slice n-1 are emitted after slice n's projections so ACT softmax work
    stays continuously fed underneath PE matmul work.
  - QK^T runs as 2 float32r matmul passes per tile instead of the hardware
    fp32 mode's 4 half-speed passes: split q = qh + eq, k = kh + ek with
    fp32r rounding (11-bit-mantissa hi; the residual is exact, qh+eq == q).
    Pass 1 (K=66): qh*kh plus two mask rows folding BOTH mask penalties
    into the matmul: q_aug = [qh, -M, 1], k_aug = [kh, 1, -M] with
    M = 2^30 (fp32r-exact; any huge exactly-representable value reproduces
    the reference's +-1e9 saturation since exp flushes it to 0).
    Pass 2 (K=128): [eq; qh] x [kh; ek] = eq*kh + qh*ek. The dropped eq*ek
    term is ~2^-26 relative. The f32 absorption (sigma - 2^30 rounds to
    exactly -2^30) reproduces the reference's masked-row arithmetic.
  - DVE output partition base may differ from its input base, but two SBUF
    inputs must share a base; head 1's packed outputs (base 64) are handled
    with base-64 temporaries and base-shifting copies.
  - softmax without a max-reduction: the reference's row-max equals the row
    penalty (masked rows: -M; unmasked rows: any shift works since scores
    are O(10)), so one ACT pass computes exp(x + M_row) with a
    per-partition bias, with accum_out producing row sums for free.
  - DVE: reciprocal of sums + per-row scale; DMA result tiles out.
"""

import os
import sys

import numpy as np

if not os.path.isdir(os.path.join(os.path.dirname(__file__), "concourse")):
    for _p in ("/opt/trn_rl_repo",):
        if os.path.isdir(_p) and _p not in sys.path:
            sys.path.insert(0, _p)

B, S, D, H = 2, 2048, 768, 12
DK = D // H  # 64
HPC = 3  # heads per core
N_CORES = 8
NQ = S // 128  # 16 query tiles per head
NKC = D // 128  # 6 contraction chunks for the projections
NN = S // 512  # 4 free-dim chunks of 512

SENTINEL = np.float32(-10000.0)
BIG = np.float32(2.0**30)

_NC = None
LAST_RESULTS = None


def _build_program():
    import concourse.bacc as bacc
    import concourse.mybir as mybir
    import concourse.tile as tile

    f32 = mybir.dt.float32
    f32r = mybir.dt.float32r
    AF = mybir.ActivationFunctionType

    nc = bacc.Bacc(
        "TRN2", target_bir_lowering=False, debug=False, enable_asserts=False
    )

    xqT = nc.dram_tensor("xqT", [D, S], f32, kind="ExternalInput").ap()
    xkT = nc.dram_tensor("xkT", [D, S], f32, kind="ExternalInput").ap()
    wq3 = nc.dram_tensor("wq3", [D, HPC * DK], f32, kind="ExternalInput").ap()
    wk3 = nc.dram_tensor("wk3", [D, HPC * DK], f32, kind="ExternalInput").ap()
    # packed biases: col 0 = [b_h0; b_h1] (128), col 1 = [b_h2; zeros]
    bqs = nc.dram_tensor("bqs", [128, 2], f32, kind="ExternalInput").ap()
    bks = nc.dram_tensor("bks", [128, 2], f32, kind="ExternalInput").ap()
    # maskaux rows: [0] = -M', [1] = ones, [2] = ones, [3] = -M'
    maskaux = nc.dram_tensor("maskaux", [4, S], f32, kind="ExternalInput").ap()
    # mrow[p, i] = M'[i*128 + p]: per-query-row exp bias
    mrow = nc.dram_tensor("mrow", [128, NQ], f32, kind="ExternalInput").ap()
    out = nc.dram_tensor("out", [HPC, S, S], f32, kind="ExternalOutput").ap()

    with tile.TileContext(nc) as tc:
        with (
            tc.tile_pool(name="const", bufs=1) as const,
            tc.tile_pool(name="aug", bufs=1) as aug,
            tc.tile_pool(name="psum", bufs=2, space="PSUM") as psum,
            tc.tile_pool(name="xio", bufs=2) as xio,
            tc.tile_pool(name="mast", bufs=1) as mast,
            tc.tile_pool(name="qm", bufs=2) as qmp,
            tc.tile_pool(name="tmp", bufs=2) as tmp,
            tc.tile_pool(name="work", bufs=3) as work,
            tc.tile_pool(name="stat", bufs=4) as stat,
        ):
            # --- K-side constants first: the first matmul only needs these.
            # Fused 3D-AP loads keep the SP issue count low at startup.
            wk_sb = const.tile([128, NKC, HPC * DK], f32, tag="wk", name="wk_sb")
            bk_sb = const.tile([128, 2], f32, tag="bk", name="bk_sb")
            nc.sync.dma_start(
                out=wk_sb, in_=wk3.rearrange("(c p) m -> p c m", p=128)
            )
            nc.sync.dma_start(out=bk_sb, in_=bks)

            # first K input slice, as one fused DMA
            def load_slice(xt, n):
                xn = xio.tile([128, NKC, 512], f32, tag="xn", name="xn")
                ns = slice(n * 512, (n + 1) * 512)
                nc.sync.dma_start(
                    out=xn, in_=xt[:, ns].rearrange("(c p) s -> p c s", p=128)
                )
                return xn

            xk_next = load_slice(xkT, 0)

            # remaining constants (overlap with k-phase compute)
            wq_sb = const.tile([128, NKC, HPC * DK], f32, tag="wq", name="wq_sb")
            bq_sb = const.tile([128, 2], f32, tag="bq", name="bq_sb")
            mrow_sb = const.tile([128, NQ], f32, tag="mrow", name="mrow_sb")
            stgq_sb = const.tile([2, S], f32, tag="stgq", name="stgq_sb")
            stgk_sb = const.tile([2, S], f32, tag="stgk", name="stgk_sb")
            nc.sync.dma_start(out=bq_sb, in_=bqs)
            nc.sync.dma_start(out=mrow_sb, in_=mrow)
            nc.sync.dma_start(out=stgq_sb, in_=maskaux[0:2, :])
            nc.sync.dma_start(out=stgk_sb, in_=maskaux[2:4, :])

            # Per-head score operands (float32r):
            # q66 = [qh(64); -M'(1); ones(1)], qcat = [eq(64); qh(64)]
            # k66 = [kh(64); ones(1); -M'(1)], kcat = [kh(64); ek(64)]
            q66s = [aug.tile([66, S], f32r, tag=f"q66_{j}", name=f"q66_{j}") for j in range(HPC)]
            k66s = [aug.tile([66, S], f32r, tag=f"k66_{j}", name=f"k66_{j}") for j in range(HPC)]
            qcats = [aug.tile([128, S], f32r, tag=f"qc_{j}", name=f"qc_{j}") for j in range(HPC)]
            kcats = [aug.tile([128, S], f32r, tag=f"kc_{j}", name=f"kc_{j}") for j in range(HPC)]
            for j in range(HPC):
                nc.vector.tensor_copy(out=q66s[j][DK : DK + 2, :], in_=stgq_sb)
                nc.vector.tensor_copy(out=k66s[j][DK : DK + 2, :], in_=stgk_sb)

            def proj_mms(wa, xn):
                """Packed projections for one input slice; one psum tile:
                cols 0-511 = heads 0+1 (M=128), cols 512-1023 = head 2."""
                pp = psum.tile([128, 1024], f32, tag="ps", name="pp")
                for c in range(NKC):
                    nc.tensor.matmul(
                        pp[:, 0:512], lhsT=wa[:, c, 0:128], rhs=xn[:, c, :],
                        start=(c == 0), stop=(c == NKC - 1),
                    )
                for c in range(NKC):
                    nc.tensor.matmul(
                        pp[0:DK, 512:1024], lhsT=wa[:, c, 128:192], rhs=xn[:, c, :],
                        start=(c == 0), stop=(c == NKC - 1),
                    )
                return pp[:, 0:512], pp[0:DK, 512:1024]

            def split_side(m01, m2, t66s, tcats, ns, qside):
                """f32r hi/lo splits for one side's slice [.., ns].

                m01: [128, 512] fp32 (h0 rows 0-63, h1 rows 64-127)
                m2:  [64, 512] fp32 (h2)
                """
                for j, msrc in ((0, m01[0:DK, :]), (2, m2[:, :])):
                    hi = t66s[j][0:DK, ns]
                    nc.vector.tensor_copy(out=hi, in_=msrc)
                    if qside:
                        # qcat = [eq(0-63); qh(64-127)]
                        nc.vector.tensor_sub(
                            out=tcats[j][0:DK, ns], in0=msrc, in1=hi.bitcast(f32)
                        )
                        nc.vector.tensor_copy(out=tcats[j][DK:128, ns], in_=hi)
                    else:
                        # kcat = [kh(0-63); ek(64-127)]
                        nc.vector.tensor_copy(out=tcats[j][0:DK, ns], in_=hi)
                        nc.vector.tensor_sub(
                            out=tcats[j][DK:128, ns], in0=msrc, in1=hi.bitcast(f32)
                        )
                # h1 lives at base 64: round there, then base-shift copies
                if qside:
                    nc.vector.tensor_copy(
                        out=tcats[1][DK:128, ns], in_=m01[DK:128, :]
                    )
                    hi1 = tcats[1][DK:128, ns]
                    nc.vector.tensor_copy(out=t66s[1][0:DK, ns], in_=hi1)
                    nc.vector.tensor_sub(
                        out=tcats[1][0:DK, ns], in0=m01[DK:128, :],
                        in1=hi1.bitcast(f32),
                    )
                else:
                    ht = tmp.tile([128, 512], f32r, tag="ht", name="ht")
                    hi1 = ht[DK:128, :]
                    nc.vector.tensor_copy(out=hi1, in_=m01[DK:128, :])
                    nc.vector.tensor_copy(out=t66s[1][0:DK, ns], in_=hi1)
                    nc.vector.tensor_copy(out=tcats[1][0:DK, ns], in_=hi1)
                    nc.vector.tensor_sub(
                        out=tcats[1][DK:128, ns], in0=m01[DK:128, :],
                        in1=hi1.bitcast(f32),
                    )

            # ---- K-side: projections + per-slice splits ----
            for n in range(NN):
                ns = slice(n * 512, (n + 1) * 512)
                xn = xk_next
                if n + 1 < NN:
                    xk_next = load_slice(xkT, n + 1)
                p01, p2 = proj_mms(wk_sb, xn)
                m01 = qmp.tile([128, 512], f32, tag="m01", name="km01")
                m2 = qmp.tile([DK, 512], f32, tag="m2", name="km2")
                nc.scalar.activation(
                    out=m01, in_=p01, func=AF.Identity, bias=bk_sb[:, 0:1], scale=1.0
                )
                nc.scalar.activation(
                    out=m2, in_=p2, func=AF.Identity, bias=bk_sb[0:DK, 1:2], scale=1.0
                )
                split_side(m01, m2, k66s, kcats, ns, qside=False)

            # load wq during the k-phase tail
            nc.sync.dma_start(
                out=wq_sb, in_=wq3.rearrange("(c p) m -> p c m", p=128)
            )

            def scores_for_slice(n):
                for j in range(HPC):
                    q66, k66, qcat, kcat = q66s[j], k66s[j], qcats[j], kcats[j]
                    for qi in range(4 * n, 4 * n + 4):
                        qs = slice(qi * 128, (qi + 1) * 128)
                        ps = psum.tile([128, S], f32, tag="ps", name="ps_sc")
                        for m in range(NN):
                            ms = slice(m * 512, (m + 1) * 512)
                            nc.tensor.matmul(
                                ps[:, ms], lhsT=q66[:, qs], rhs=k66[:, ms],
                                start=True, stop=False,
                            )
                            nc.tensor.matmul(
                                ps[:, ms], lhsT=qcat[:, qs], rhs=kcat[:, ms],
                                start=False, stop=True,
                            )
                        ex = work.tile([128, S], f32, tag="ex", name="ex")
                        rs = stat.tile([128, 1], f32, tag="rs", name="rs")
                        nc.scalar.activation(
                            out=ex, in_=ps, func=AF.Exp,
                            bias=mrow_sb[:, qi : qi + 1], scale=1.0,
                            accum_out=rs,
                        )
                        nc.vector.reciprocal(rs, rs)
                        nc.vector.tensor_scalar_mul(out=ex, in0=ex, scalar1=rs)
                        nc.sync.dma_start(out=out[j, qs, :], in_=ex)

            # ---- Q-side per slice; scores pipeline-shifted by one slice ----
            for n in range(NN):
                ns = slice(n * 512, (n + 1) * 512)
                xn = load_slice(xqT, n)
                q01, q2 = proj_mms(wq_sb, xn)
                qm01 = qmp.tile([128, 512], f32, tag="m01", name="qm01")
                qm2 = qmp.tile([DK, 512], f32, tag="m2", name="qm2")
                nc.scalar.activation(
                    out=qm01, in_=q01, func=AF.Identity, bias=bq_sb[:, 0:1], scale=1.0
                )
                nc.scalar.activation(
                    out=qm2, in_=q2, func=AF.Identity, bias=bq_sb[0:DK, 1:2], scale=1.0
                )
                split_side(qm01, qm2, q66s, qcats, ns, qside=True)
                scores_for_slice(n)

    nc.compile()
    return nc


def _get_program():
    global _NC
    if _NC is None:
        _NC = _build_program()
    return _NC


def kernel(query, key, mask, Wq, bq, Wk, bk):
    global LAST_RESULTS
    from concourse.bass_utils import run_bass_kernel_spmd

    query = np.asarray(query, dtype=np.float32)
    key = np.asarray(key, dtype=np.float32)
    mask = np.asarray(mask, dtype=np.float32)
    Wq = np.asarray(Wq, dtype=np.float32)
    bq = np.asarray(bq, dtype=np.float32)
    Wk = np.asarray(Wk, dtype=np.float32)
    bk = np.asarray(bk, dtype=np.float32)

    nc = _get_program()

    ones_row = np.ones(S, dtype=np.float32)
    zeros64 = np.zeros(DK, dtype=np.float32)
    in_maps = []
    for core in range(N_CORES):
        b = core // 4
        g = core % 4
        rows = slice(g * HPC * DK, (g + 1) * HPC * DK)
        mprime = np.where(mask[b] == SENTINEL, BIG, np.float32(0.0)).astype(
            np.float32
        )
        neg_m = -mprime
        bq3 = bq[rows] * np.float32(0.125)  # [192]
        bk3 = bk[rows]
        in_maps.append(
            {
                "xqT": np.ascontiguousarray(query[b].T),
                "xkT": np.ascontiguousarray(key[b].T),
                "wq3": np.ascontiguousarray(Wq[rows].T) * np.float32(0.125),
                "wk3": np.ascontiguousarray(Wk[rows].T),
                "bqs": np.ascontiguousarray(
                    np.stack(
                        [bq3[0:128], np.concatenate([bq3[128:192], zeros64])],
                        axis=1,
                    )
                ),
                "bks": np.ascontiguousarray(
                    np.stack(
                        [bk3[0:128], np.concatenate([bk3[128:192], zeros64])],
                        axis=1,
                    )
                ),
                "maskaux": np.ascontiguousarray(
                    np.stack([neg_m, ones_row, ones_row, neg_m])
                ),
                "mrow": np.ascontiguousarray(mprime.reshape(NQ, 128).T),
            }
        )

    trace = os.environ.get("BASS_KERNEL_TRACE") == "1"
    res = run_bass_kernel_spmd(
        nc, in_maps, core_ids=list(range(N_CORES)), trace=trace
    )
    LAST_RESULTS = res
    outs = np.stack([res.results[c]["out"] for c in range(N_CORES)])
    return outs.reshape(B, H, S, S)


# revision 25
# speedup vs baseline: 1.1416x; 1.1416x over previous
"""Trainium2 Bass kernel for masked multi-head attention scores (softmax(QK^T)).

Reference computation (B=2, S=2048, D=768, H=12, DK=64):
    q = (query @ Wq.T + bq)  -> [B,H,S,DK]
    k = (key   @ Wk.T + bk)  -> [B,H,S,DK]
    scores = q @ k.T / sqrt(DK)            [B,H,S,S]
    m = where(mask == -10000, 1e9, 0)      [B,S]
    scores = scores - m[:,None,:,None] - m[:,None,None,:]
    out = softmax(scores, axis=-1)

Sharding: 8 cores = 2 batches x 4 head-groups (3 heads each). Each core gets
its batch's query^T/key^T (pre-transposed on host - pure layout change), its
3 heads' weight slices, and computes softmax scores for those heads.

Device algorithm per core:
  - Projections into [dk, s] layout via PE matmuls; heads 0+1 are packed
    into one M=128 matmul (they share the same rhs); 1/sqrt(DK) is folded
    into Wq/bq as an exact power-of-2 scale. K side runs first (scores need
    all of k); the Q side streams per 512-column slice, and the scores for
    slice n-1 are emitted after slice n's projections so ACT softmax work
    stays continuously fed underneath PE matmul work.
  - QK^T runs as 2 float32r matmul passes per tile instead of the hardware
    fp32 mode's 4 half-speed passes: split q = qh + eq, k = kh + ek with
    fp32r rounding (11-bit-mantissa hi; the residual is exact, qh+eq == q).
    Pass 1 (K=66): qh*kh plus two mask rows folding BOTH mask penalties
    into the matmul: q_aug = [qh, -M, 1], k_aug = [kh, 1, -M] with
    M = 2^30 (fp32r-exact; any huge exactly-representable value reproduces
    the reference's +-1e9 saturation since exp flushes it to 0).
    Pass 2 (K=128): [eq; qh] x [kh; ek] = eq*kh + qh*ek. The dropped eq*ek
    term is ~2^-26 relative. The f32 absorption (sigma - 2^30 rounds to
    exactly -2^30) reproduces the reference's masked-row arithmetic.
  - DVE output partition base may differ from its input base, but two SBUF
    inputs must share a base; head 1's packed outputs (base 64) are handled
    with base-64 temporaries and base-shifting copies.
  - softmax without a max-reduction: the reference's row-max equals the row
    penalty (masked rows: -M; unmasked rows: any shift works since scores
    are O(10)), so one ACT pass computes exp(x + M_row) with a
    per-partition bias, with accum_out producing row sums for free.
  - DVE: reciprocal of sums + per-row scale; DMA result tiles out.
"""

import os
import sys

import numpy as np

if not os.path.isdir(os.path.join(os.path.dirname(__file__), "concourse")):
    for _p in ("/opt/trn_rl_repo",):
        if os.path.isdir(_p) and _p not in sys.path:
            sys.path.insert(0, _p)

B, S, D, H = 2, 2048, 768, 12
DK = D // H  # 64
HPC = 3  # heads per core
N_CORES = 8
NQ = S // 128  # 16 query tiles per head
NKC = D // 128  # 6 contraction chunks for the projections
NN = S // 512  # 4 free-dim chunks of 512

SENTINEL = np.float32(-10000.0)
BIG = np.float32(2.0**30)

_NC = None
LAST_RESULTS = None


def _build_program():
    import concourse.bacc as bacc
    import concourse.mybir as mybir
    import concourse.tile as tile

    f32 = mybir.dt.float32
    f32r = mybir.dt.float32r
    AF = mybir.ActivationFunctionType

    nc = bacc.Bacc(
        "TRN2", target_bir_lowering=False, debug=False, enable_asserts=False
    )

    xqT = nc.dram_tensor("xqT", [D, S], f32, kind="ExternalInput").ap()
    xkT = nc.dram_tensor("xkT", [D, S], f32, kind="ExternalInput").ap()
    wq3 = nc.dram_tensor("wq3", [D, HPC * DK], f32, kind="ExternalInput").ap()
    wk3 = nc.dram_tensor("wk3", [D, HPC * DK], f32, kind="ExternalInput").ap()
    # packed biases: col 0 = [b_h0; b_h1] (128), col 1 = [b_h2; zeros]
    bqs = nc.dram_tensor("bqs", [128, 2], f32, kind="ExternalInput").ap()
    bks = nc.dram_tensor("bks", [128, 2], f32, kind="ExternalInput").ap()
    # maskaux rows: [0] = -M', [1] = ones, [2] = ones, [3] = -M'
    maskaux = nc.dram_tensor("maskaux", [4, S], f32, kind="ExternalInput").ap()
    # mrow[p, i] = M'[i*128 + p]: per-query-row exp bias
    mrow = nc.dram_tensor("mrow", [128, NQ], f32, kind="ExternalInput").ap()
    out = nc.dram_tensor("out", [HPC, S, S], f32, kind="ExternalOutput").ap()

    with tile.TileContext(nc) as tc:
        with (
            tc.tile_pool(name="const", bufs=1) as const,
            tc.tile_pool(name="aug", bufs=1) as aug,
            tc.tile_pool(name="psum", bufs=2, space="PSUM") as psum,
            tc.tile_pool(name="xio", bufs=2) as xio,
            tc.tile_pool(name="mast", bufs=1) as mast,
            tc.tile_pool(name="qm", bufs=2) as qmp,
            tc.tile_pool(name="tmp", bufs=2) as tmp,
            tc.tile_pool(name="work", bufs=3) as work,
            tc.tile_pool(name="stat", bufs=4) as stat,
        ):
            # --- K-side constants first: the first matmul only needs these.
            # Fused 3D-AP loads keep the SP issue count low at startup.
            wk_sb = const.tile([128, NKC, HPC * DK], f32, tag="wk", name="wk_sb")
            bk_sb = const.tile([128, 2], f32, tag="bk", name="bk_sb")
            nc.sync.dma_start(
                out=wk_sb, in_=wk3.rearrange("(c p) m -> p c m", p=128)
            )
            nc.sync.dma_start(out=bk_sb, in_=bks)

            # first K input slice, as one fused DMA
            def load_slice(xt, n):
                xn = xio.tile([128, NKC, 512], f32, tag="xn", name="xn")
                ns = slice(n * 512, (n + 1) * 512)
                nc.sync.dma_start(
                    out=xn, in_=xt[:, ns].rearrange("(c p) s -> p c s", p=128)
                )
                return xn

            xk_next = load_slice(xkT, 0)

            # remaining constants (overlap with k-phase compute)
            wq_sb = const.tile([128, NKC, HPC * DK], f32, tag="wq", name="wq_sb")
            bq_sb = const.tile([128, 2], f32, tag="bq", name="bq_sb")
            mrow_sb = const.tile([128, NQ], f32, tag="mrow", name="mrow_sb")
            stgq_sb = const.tile([2, S], f32, tag="stgq", name="stgq_sb")
            stgk_sb = const.tile([2, S], f32, tag="stgk", name="stgk_sb")
            nc.sync.dma_start(out=bq_sb, in_=bqs)
            nc.sync.dma_start(out=mrow_sb, in_=mrow)
            nc.sync.dma_start(out=stgq_sb, in_=maskaux[0:2, :])
            nc.sync.dma_start(out=stgk_sb, in_=maskaux[2:4, :])

            # Per-head score operands (float32r):
            # q66 = [qh(64); -M'(1); ones(1)], qcat = [eq(64); qh(64)]
            # k66 = [kh(64); ones(1); -M'(1)], kcat = [kh(64); ek(64)]
            q66s = [aug.tile([66, S], f32r, tag=f"q66_{j}", name=f"q66_{j}") for j in range(HPC)]
            k66s = [aug.tile([66, S], f32r, tag=f"k66_{j}", name=f"k66_{j}") for j in range(HPC)]
            qcats = [aug.tile([128, S], f32r, tag=f"qc_{j}", name=f"qc_{j}") for j in range(HPC)]
            kcats = [aug.tile([128, S], f32r, tag=f"kc_{j}", name=f"kc_{j}") for j in range(HPC)]
            for j in range(HPC):
                nc.vector.tensor_copy(out=q66s[j][DK : DK + 2, :], in_=stgq_sb)
                nc.vector.tensor_copy(out=k66s[j][DK : DK + 2, :], in_=stgk_sb)

            def proj_mms(wa, xn):
                """Packed projections for one input slice -> (p01, p2) psums."""
                p01 = psum.tile([128, 512], f32, tag="ps", name="p01")
                for c in range(NKC):
                    nc.tensor.matmul(
                        p01, lhsT=wa[:, c, 0:128], rhs=xn[:, c, :],
                        start=(c == 0), stop=(c == NKC - 1),
                    )
                p2 = psum.tile([DK, 512], f32, tag="ps", name="p2")
                for c in range(NKC):
                    nc.tensor.matmul(
                        p2, lhsT=wa[:, c, 128:192], rhs=xn[:, c, :],
                        start=(c == 0), stop=(c == NKC - 1),
                    )
                return p01, p2

            def split_side(m01, m2, t66s, tcats, ns, qside):
                """f32r hi/lo splits for one side's slice [.., ns].

                m01: [128, 512] fp32 (h0 rows 0-63, h1 rows 64-127)
                m2:  [64, 512] fp32 (h2)
                """
                for j, msrc in ((0, m01[0:DK, :]), (2, m2[:, :])):
                    hi = t66s[j][0:DK, ns]
                    nc.vector.tensor_copy(out=hi, in_=msrc)
                    if qside:
                        # qcat = [eq(0-63); qh(64-127)]
                        nc.vector.tensor_sub(
                            out=tcats[j][0:DK, ns], in0=msrc, in1=hi.bitcast(f32)
                        )
                        nc.vector.tensor_copy(out=tcats[j][DK:128, ns], in_=hi)
                    else:
                        # kcat = [kh(0-63); ek(64-127)]
                        nc.vector.tensor_copy(out=tcats[j][0:DK, ns], in_=hi)
                        nc.vector.tensor_sub(
                            out=tcats[j][DK:128, ns], in0=msrc, in1=hi.bitcast(f32)
                        )
                # h1 lives at base 64: round there, then base-shift copies
                if qside:
                    nc.vector.tensor_copy(
                        out=tcats[1][DK:128, ns], in_=m01[DK:128, :]
                    )
                    hi1 = tcats[1][DK:128, ns]
                    nc.vector.tensor_copy(out=t66s[1][0:DK, ns], in_=hi1)
                    nc.vector.tensor_sub(
                        out=tcats[1][0:DK, ns], in0=m01[DK:128, :],
                        in1=hi1.bitcast(f32),
                    )
                else:
                    ht = tmp.tile([128, 512], f32r, tag="ht", name="ht")
                    hi1 = ht[DK:128, :]
                    nc.vector.tensor_copy(out=hi1, in_=m01[DK:128, :])
                    nc.vector.tensor_copy(out=t66s[1][0:DK, ns], in_=hi1)
                    nc.vector.tensor_copy(out=tcats[1][0:DK, ns], in_=hi1)
                    nc.vector.tensor_sub(
                        out=tcats[1][DK:128, ns], in0=m01[DK:128, :],
                        in1=hi1.bitcast(f32),
                    )

            # ---- K-side: projections + per-slice splits ----
            for n in range(NN):
                ns = slice(n * 512, (n + 1) * 512)
                xn = xk_next
                if n + 1 < NN:
                    xk_next = load_slice(xkT, n + 1)
                p01, p2 = proj_mms(wk_sb, xn)
                m01 = qmp.tile([128, 512], f32, tag="m01", name="km01")
                m2 = qmp.tile([DK, 512], f32, tag="m2", name="km2")
                nc.scalar.activation(
                    out=m01, in_=p01, func=AF.Identity, bias=bk_sb[:, 0:1], scale=1.0
                )
                nc.scalar.activation(
                    out=m2, in_=p2, func=AF.Identity, bias=bk_sb[0:DK, 1:2], scale=1.0
                )
                split_side(m01, m2, k66s, kcats, ns, qside=False)

            # load wq during the k-phase tail
            nc.sync.dma_start(
                out=wq_sb, in_=wq3.rearrange("(c p) m -> p c m", p=128)
            )

            def scores_for_slice(n):
                for j in range(HPC):
                    q66, k66, qcat, kcat = q66s[j], k66s[j], qcats[j], kcats[j]
                    for qi in range(4 * n, 4 * n + 4):
                        qs = slice(qi * 128, (qi + 1) * 128)
                        ps = psum.tile([128, S], f32, tag="ps", name="ps_sc")
                        for m in range(NN):
                            ms = slice(m * 512, (m + 1) * 512)
                            nc.tensor.matmul(
                                ps[:, ms], lhsT=q66[:, qs], rhs=k66[:, ms],
                                start=True, stop=False,
                            )
                            nc.tensor.matmul(
                                ps[:, ms], lhsT=qcat[:, qs], rhs=kcat[:, ms],
                                start=False, stop=True,
                            )
                        ex = work.tile([128, S], f32, tag="ex", name="ex")
                        rs = stat.tile([128, 1], f32, tag="rs", name="rs")
                        nc.scalar.activation(
                            out=ex, in_=ps, func=AF.Exp,
                            bias=mrow_sb[:, qi : qi + 1], scale=1.0,
                            accum_out=rs,
                        )
                        nc.vector.reciprocal(rs, rs)
                        nc.vector.tensor_scalar_mul(out=ex, in0=ex, scalar1=rs)
                        nc.sync.dma_start(out=out[j, qs, :], in_=ex)

            # ---- Q-side per slice; scores pipeline-shifted by one slice ----
            for n in range(NN):
                ns = slice(n * 512, (n + 1) * 512)
                xn = load_slice(xqT, n)
                q01, q2 = proj_mms(wq_sb, xn)
                qm01 = qmp.tile([128, 512], f32, tag="m01", name="qm01")
                qm2 = qmp.tile([DK, 512], f32, tag="m2", name="qm2")
                nc.scalar.activation(
                    out=qm01, in_=q01, func=AF.Identity, bias=bq_sb[:, 0:1], scale=1.0
                )
                nc.scalar.activation(
                    out=qm2, in_=q2, func=AF.Identity, bias=bq_sb[0:DK, 1:2], scale=1.0
                )
                split_side(qm01, qm2, q66s, qcats, ns, qside=True)
                scores_for_slice(n)

    nc.compile()
    return nc


def _get_program():
    global _NC
    if _NC is None:
        _NC = _build_program()
    return _NC


def kernel(query, key, mask, Wq, bq, Wk, bk):
    global LAST_RESULTS
    from concourse.bass_utils import run_bass_kernel_spmd

    query = np.asarray(query, dtype=np.float32)
    key = np.asarray(key, dtype=np.float32)
    mask = np.asarray(mask, dtype=np.float32)
    Wq = np.asarray(Wq, dtype=np.float32)
    bq = np.asarray(bq, dtype=np.float32)
    Wk = np.asarray(Wk, dtype=np.float32)
    bk = np.asarray(bk, dtype=np.float32)

    nc = _get_program()

    ones_row = np.ones(S, dtype=np.float32)
    zeros64 = np.zeros(DK, dtype=np.float32)
    in_maps = []
    for core in range(N_CORES):
        b = core // 4
        g = core % 4
        rows = slice(g * HPC * DK, (g + 1) * HPC * DK)
        mprime = np.where(mask[b] == SENTINEL, BIG, np.float32(0.0)).astype(
            np.float32
        )
        neg_m = -mprime
        bq3 = bq[rows] * np.float32(0.125)  # [192]
        bk3 = bk[rows]
        in_maps.append(
            {
                "xqT": np.ascontiguousarray(query[b].T),
                "xkT": np.ascontiguousarray(key[b].T),
                "wq3": np.ascontiguousarray(Wq[rows].T) * np.float32(0.125),
                "wk3": np.ascontiguousarray(Wk[rows].T),
                "bqs": np.ascontiguousarray(
                    np.stack(
                        [bq3[0:128], np.concatenate([bq3[128:192], zeros64])],
                        axis=1,
                    )
                ),
                "bks": np.ascontiguousarray(
                    np.stack(
                        [bk3[0:128], np.concatenate([bk3[128:192], zeros64])],
                        axis=1,
                    )
                ),
                "maskaux": np.ascontiguousarray(
                    np.stack([neg_m, ones_row, ones_row, neg_m])
                ),
                "mrow": np.ascontiguousarray(mprime.reshape(NQ, 128).T),
            }
        )

    trace = os.environ.get("BASS_KERNEL_TRACE") == "1"
    res = run_bass_kernel_spmd(
        nc, in_maps, core_ids=list(range(N_CORES)), trace=trace
    )
    LAST_RESULTS = res
    outs = np.stack([res.results[c]["out"] for c in range(N_CORES)])
    return outs.reshape(B, H, S, S)


# revision 28
# speedup vs baseline: 1.2326x; 1.0797x over previous
"""Trainium2 Bass kernel for masked multi-head attention scores (softmax(QK^T)).

Reference computation (B=2, S=2048, D=768, H=12, DK=64):
    q = (query @ Wq.T + bq)  -> [B,H,S,DK]
    k = (key   @ Wk.T + bk)  -> [B,H,S,DK]
    scores = q @ k.T / sqrt(DK)            [B,H,S,S]
    m = where(mask == -10000, 1e9, 0)      [B,S]
    scores = scores - m[:,None,:,None] - m[:,None,None,:]
    out = softmax(scores, axis=-1)

Sharding: 8 cores = 2 batches x 4 head-groups (3 heads each). Each core gets
its batch's query^T/key^T (pre-transposed on host - pure layout change), its
3 heads' weight slices, and computes softmax scores for those heads.

Device algorithm per core:
  - Projections into [dk, s] layout via PE matmuls; heads 0+1 are packed
    into one M=128 matmul (they share the same rhs); 1/sqrt(DK) is folded
    into Wq/bq as an exact power-of-2 scale. K side runs first (scores need
    all of k); the Q side streams per 512-column slice, and the scores for
    slice n-1 are emitted after slice n's projections so ACT softmax work
    stays continuously fed underneath PE matmul work.
  - QK^T runs as 2 float32r matmul passes per tile instead of the hardware
    fp32 mode's 4 half-speed passes: split q = qh + eq, k = kh + ek with
    fp32r rounding (11-bit-mantissa hi; the residual is exact, qh+eq == q).
    Pass 1 (K=66): qh*kh plus two mask rows folding BOTH mask penalties
    into the matmul: q_aug = [qh, -M, 1], k_aug = [kh, 1, -M] with
    M = 2^30 (fp32r-exact; any huge exactly-representable value reproduces
    the reference's +-1e9 saturation since exp flushes it to 0).
    Pass 2 (K=128): [eq; qh] x [kh; ek] = eq*kh + qh*ek. The dropped eq*ek
    term is ~2^-26 relative. The f32 absorption (sigma - 2^30 rounds to
    exactly -2^30) reproduces the reference's masked-row arithmetic.
  - DVE output partition base may differ from its input base, but two SBUF
    inputs must share a base; head 1's packed outputs (base 64) are handled
    with base-64 temporaries and base-shifting copies.
  - softmax without a max-reduction: the reference's row-max equals the row
    penalty (masked rows: -M; unmasked rows: any shift works since scores
    are O(10)), so one ACT pass computes exp(x + M_row) with a
    per-partition bias, with accum_out producing row sums for free.
  - DVE: reciprocal of sums + per-row scale; DMA result tiles out.
"""

import os
import sys

import numpy as np

if not os.path.isdir(os.path.join(os.path.dirname(__file__), "concourse")):
    for _p in ("/opt/trn_rl_repo",):
        if os.path.isdir(_p) and _p not in sys.path:
            sys.path.insert(0, _p)

B, S, D, H = 2, 2048, 768, 12
DK = D // H  # 64
HPC = 3  # heads per core
N_CORES = 8
NQ = S // 128  # 16 query tiles per head
NKC = D // 128  # 6 contraction chunks for the projections
NN = S // 512  # 4 free-dim chunks of 512

SENTINEL = np.float32(-10000.0)
BIG = np.float32(2.0**30)

_NC = None
LAST_RESULTS = None


def _build_program():
    import concourse.bacc as bacc
    import concourse.mybir as mybir
    import concourse.tile as tile

    f32 = mybir.dt.float32
    f32r = mybir.dt.float32r
    AF = mybir.ActivationFunctionType

    nc = bacc.Bacc(
        "TRN2", target_bir_lowering=False, debug=False, enable_asserts=False
    )

    xqT = nc.dram_tensor("xqT", [D, S], f32, kind="ExternalInput").ap()
    xkT = nc.dram_tensor("xkT", [D, S], f32, kind="ExternalInput").ap()
    wq3 = nc.dram_tensor("wq3", [D, HPC * DK], f32, kind="ExternalInput").ap()
    wk3 = nc.dram_tensor("wk3", [D, HPC * DK], f32, kind="ExternalInput").ap()
    # packed biases: col 0 = [b_h0; b_h1] (128), col 1 = [b_h2; zeros]
    bqs = nc.dram_tensor("bqs", [128, 2], f32, kind="ExternalInput").ap()
    bks = nc.dram_tensor("bks", [128, 2], f32, kind="ExternalInput").ap()
    # maskaux rows: [0] = -M', [1] = ones, [2] = ones, [3] = -M'
    maskaux = nc.dram_tensor("maskaux", [4, S], f32, kind="ExternalInput").ap()
    # mrow[p, i] = M'[i*128 + p]: per-query-row exp bias
    mrow = nc.dram_tensor("mrow", [128, NQ], f32, kind="ExternalInput").ap()
    out = nc.dram_tensor("out", [HPC, S, S], f32, kind="ExternalOutput").ap()

    with tile.TileContext(nc) as tc:
        with (
            tc.tile_pool(name="const", bufs=1) as const,
            tc.tile_pool(name="aug", bufs=1) as aug,
            tc.tile_pool(name="psum", bufs=2, space="PSUM") as psum,
            tc.tile_pool(name="xio", bufs=2) as xio,
            tc.tile_pool(name="mast", bufs=1) as mast,
            tc.tile_pool(name="qm", bufs=2) as qmp,
            tc.tile_pool(name="tmp", bufs=2) as tmp,
            tc.tile_pool(name="work", bufs=3) as work,
            tc.tile_pool(name="stat", bufs=4) as stat,
        ):
            # --- K-side constants first: the first matmul only needs these.
            # Fused 3D-AP loads keep the SP issue count low at startup.
            wk_sb = const.tile([128, NKC, HPC * DK], f32, tag="wk", name="wk_sb")
            bk_sb = const.tile([128, 2], f32, tag="bk", name="bk_sb")

            # first K input slice, in two fused halves so the first matmuls
            # can start as soon as chunks 0-2 land
            def load_slice(xt, n, halves=1):
                xn = xio.tile([128, NKC, 512], f32, tag="xn", name="xn")
                ns = slice(n * 512, (n + 1) * 512)
                src_ap = xt[:, ns].rearrange("(c p) s -> p c s", p=128)
                if halves == 2:
                    h = NKC // 2
                    nc.sync.dma_start(out=xn[:, 0:h, :], in_=src_ap[:, 0:h, :])
                    nc.sync.dma_start(out=xn[:, h:NKC, :], in_=src_ap[:, h:NKC, :])
                else:
                    nc.sync.dma_start(out=xn, in_=src_ap)
                return xn

            nc.sync.dma_start(
                out=wk_sb, in_=wk3.rearrange("(c p) m -> p c m", p=128)
            )
            xk_next = load_slice(xkT, 0, halves=2)
            nc.sync.dma_start(out=bk_sb, in_=bks)

            # remaining constants (overlap with k-phase compute)
            wq_sb = const.tile([128, NKC, HPC * DK], f32, tag="wq", name="wq_sb")
            bq_sb = const.tile([128, 2], f32, tag="bq", name="bq_sb")
            mrow_sb = const.tile([128, NQ], f32, tag="mrow", name="mrow_sb")
            stgq_sb = const.tile([2, S], f32, tag="stgq", name="stgq_sb")
            stgk_sb = const.tile([2, S], f32, tag="stgk", name="stgk_sb")
            nc.sync.dma_start(out=bq_sb, in_=bqs)
            nc.sync.dma_start(out=mrow_sb, in_=mrow)
            nc.sync.dma_start(out=stgq_sb, in_=maskaux[0:2, :])
            nc.sync.dma_start(out=stgk_sb, in_=maskaux[2:4, :])

            # Per-head score operands (float32r):
            # q66 = [qh(64); -M'(1); ones(1)], qcat = [eq(64); qh(64)]
            # k66 = [kh(64); ones(1); -M'(1)], kcat = [kh(64); ek(64)]
            k66s = [aug.tile([66, S], f32r, tag=f"k66_{j}", name=f"k66_{j}") for j in range(HPC)]
            kcats = [aug.tile([128, S], f32r, tag=f"kc_{j}", name=f"kc_{j}") for j in range(HPC)]
            for j in range(HPC):
                nc.vector.tensor_copy(out=k66s[j][DK : DK + 2, :], in_=stgk_sb)

            def proj_mms(wa, xn):
                """Packed projections for one input slice -> (p01, p2) psums."""
                p01 = psum.tile([128, 512], f32, tag="ps", name="p01")
                for c in range(NKC):
                    nc.tensor.matmul(
                        p01, lhsT=wa[:, c, 0:128], rhs=xn[:, c, :],
                        start=(c == 0), stop=(c == NKC - 1),
                    )
                p2 = psum.tile([DK, 512], f32, tag="ps", name="p2")
                for c in range(NKC):
                    nc.tensor.matmul(
                        p2, lhsT=wa[:, c, 128:192], rhs=xn[:, c, :],
                        start=(c == 0), stop=(c == NKC - 1),
                    )
                return p01, p2

            def split_k(m01, m2, ns):
                """f32r hi/lo splits for a K-side slice [.., ns] into the
                persistent k66/kcat tiles."""
                for j, msrc in ((0, m01[0:DK, :]), (2, m2[:, :])):
                    hi = k66s[j][0:DK, ns]
                    nc.vector.tensor_copy(out=hi, in_=msrc)
                    nc.vector.tensor_copy(out=kcats[j][0:DK, ns], in_=hi)
                    nc.vector.tensor_sub(
                        out=kcats[j][DK:128, ns], in0=msrc, in1=hi.bitcast(f32)
                    )
                # h1 lives at base 64: round there, then base-shift copies
                ht = tmp.tile([128, 512], f32r, tag="ht", name="ht")
                hi1 = ht[DK:128, :]
                nc.vector.tensor_copy(out=hi1, in_=m01[DK:128, :])
                nc.vector.tensor_copy(out=k66s[1][0:DK, ns], in_=hi1)
                nc.vector.tensor_copy(out=kcats[1][0:DK, ns], in_=hi1)
                nc.vector.tensor_sub(
                    out=kcats[1][DK:128, ns], in0=m01[DK:128, :],
                    in1=hi1.bitcast(f32),
                )

            def split_q(m01, m2, ns):
                """f32r hi/lo splits for a Q-side slice into fresh per-slice
                [66, 512] / [128, 512] tiles (double-buffered by tag)."""
                q66t = [
                    aug.tile([66, 512], f32r, tag=f"q66_{j}", name=f"q66_{j}", bufs=2)
                    for j in range(HPC)
                ]
                qcatt = [
                    aug.tile([128, 512], f32r, tag=f"qc_{j}", name=f"qc_{j}", bufs=2)
                    for j in range(HPC)
                ]
                for j, msrc in ((0, m01[0:DK, :]), (2, m2[:, :])):
                    hi = q66t[j][0:DK, :]
                    nc.vector.tensor_copy(out=hi, in_=msrc)
                    nc.vector.tensor_sub(
                        out=qcatt[j][0:DK, :], in0=msrc, in1=hi.bitcast(f32)
                    )
                    nc.vector.tensor_copy(out=qcatt[j][DK:128, :], in_=hi)
                nc.vector.tensor_copy(out=qcatt[1][DK:128, :], in_=m01[DK:128, :])
                hi1 = qcatt[1][DK:128, :]
                nc.vector.tensor_copy(out=q66t[1][0:DK, :], in_=hi1)
                nc.vector.tensor_sub(
                    out=qcatt[1][0:DK, :], in0=m01[DK:128, :],
                    in1=hi1.bitcast(f32),
                )
                for j in range(HPC):
                    nc.vector.tensor_copy(
                        out=q66t[j][DK : DK + 2, :], in_=stgq_sb[:, ns]
                    )
                return q66t, qcatt

            # ---- K-side: projections + per-slice splits ----
            for n in range(NN):
                ns = slice(n * 512, (n + 1) * 512)
                xn = xk_next
                if n + 1 < NN:
                    xk_next = load_slice(xkT, n + 1)
                p01, p2 = proj_mms(wk_sb, xn)
                m01 = qmp.tile([128, 512], f32, tag="m01", name="km01")
                m2 = qmp.tile([DK, 512], f32, tag="m2", name="km2")
                nc.scalar.activation(
                    out=m01, in_=p01, func=AF.Identity, bias=bk_sb[:, 0:1], scale=1.0
                )
                nc.scalar.activation(
                    out=m2, in_=p2, func=AF.Identity, bias=bk_sb[0:DK, 1:2], scale=1.0
                )
                split_k(m01, m2, ns)

            # load wq during the k-phase tail
            nc.sync.dma_start(
                out=wq_sb, in_=wq3.rearrange("(c p) m -> p c m", p=128)
            )

            def scores_for_slice(n, q66t, qcatt):
                for j in range(HPC):
                    q66, k66, qcat, kcat = q66t[j], k66s[j], qcatt[j], kcats[j]
                    for qi in range(4 * n, 4 * n + 4):
                        ql = slice((qi % 4) * 128, (qi % 4 + 1) * 128)
                        qs = slice(qi * 128, (qi + 1) * 128)
                        ps = psum.tile([128, S], f32, tag="ps", name="ps_sc")
                        for m in range(NN):
                            ms = slice(m * 512, (m + 1) * 512)
                            nc.tensor.matmul(
                                ps[:, ms], lhsT=q66[:, ql], rhs=k66[:, ms],
                                start=True, stop=False,
                            )
                            nc.tensor.matmul(
                                ps[:, ms], lhsT=qcat[:, ql], rhs=kcat[:, ms],
                                start=False, stop=True,
                            )
                        ex = work.tile([128, S], f32, tag="ex", name="ex")
                        rs = stat.tile([128, 1], f32, tag="rs", name="rs")
                        nc.scalar.activation(
                            out=ex, in_=ps, func=AF.Exp,
                            bias=mrow_sb[:, qi : qi + 1], scale=1.0,
                            accum_out=rs,
                        )
                        nc.vector.reciprocal(rs, rs)
                        nc.vector.tensor_scalar_mul(out=ex, in0=ex, scalar1=rs)
                        nc.sync.dma_start(out=out[j, qs, :], in_=ex)

            # ---- Q-side per slice; scores pipeline-shifted by one slice ----
            for n in range(NN):
                ns = slice(n * 512, (n + 1) * 512)
                xn = load_slice(xqT, n)
                q01, q2 = proj_mms(wq_sb, xn)
                qm01 = qmp.tile([128, 512], f32, tag="m01", name="qm01")
                qm2 = qmp.tile([DK, 512], f32, tag="m2", name="qm2")
                nc.scalar.activation(
                    out=qm01, in_=q01, func=AF.Identity, bias=bq_sb[:, 0:1], scale=1.0
                )
                nc.scalar.activation(
                    out=qm2, in_=q2, func=AF.Identity, bias=bq_sb[0:DK, 1:2], scale=1.0
                )
                q66t, qcatt = split_q(qm01, qm2, ns)
                scores_for_slice(n, q66t, qcatt)

    nc.compile()
    return nc


def _get_program():
    global _NC
    if _NC is None:
        _NC = _build_program()
    return _NC


def kernel(query, key, mask, Wq, bq, Wk, bk):
    global LAST_RESULTS
    from concourse.bass_utils import run_bass_kernel_spmd

    query = np.asarray(query, dtype=np.float32)
    key = np.asarray(key, dtype=np.float32)
    mask = np.asarray(mask, dtype=np.float32)
    Wq = np.asarray(Wq, dtype=np.float32)
    bq = np.asarray(bq, dtype=np.float32)
    Wk = np.asarray(Wk, dtype=np.float32)
    bk = np.asarray(bk, dtype=np.float32)

    nc = _get_program()

    ones_row = np.ones(S, dtype=np.float32)
    zeros64 = np.zeros(DK, dtype=np.float32)
    in_maps = []
    for core in range(N_CORES):
        b = core // 4
        g = core % 4
        rows = slice(g * HPC * DK, (g + 1) * HPC * DK)
        mprime = np.where(mask[b] == SENTINEL, BIG, np.float32(0.0)).astype(
            np.float32
        )
        neg_m = -mprime
        bq3 = bq[rows] * np.float32(0.125)  # [192]
        bk3 = bk[rows]
        in_maps.append(
            {
                "xqT": np.ascontiguousarray(query[b].T),
                "xkT": np.ascontiguousarray(key[b].T),
                "wq3": np.ascontiguousarray(Wq[rows].T) * np.float32(0.125),
                "wk3": np.ascontiguousarray(Wk[rows].T),
                "bqs": np.ascontiguousarray(
                    np.stack(
                        [bq3[0:128], np.concatenate([bq3[128:192], zeros64])],
                        axis=1,
                    )
                ),
                "bks": np.ascontiguousarray(
                    np.stack(
                        [bk3[0:128], np.concatenate([bk3[128:192], zeros64])],
                        axis=1,
                    )
                ),
                "maskaux": np.ascontiguousarray(
                    np.stack([neg_m, ones_row, ones_row, neg_m])
                ),
                "mrow": np.ascontiguousarray(mprime.reshape(NQ, 128).T),
            }
        )

    trace = os.environ.get("BASS_KERNEL_TRACE") == "1"
    res = run_bass_kernel_spmd(
        nc, in_maps, core_ids=list(range(N_CORES)), trace=trace
    )
    LAST_RESULTS = res
    outs = np.stack([res.results[c]["out"] for c in range(N_CORES)])
    return outs.reshape(B, H, S, S)
